# revision 26
# baseline (speedup 1.0000x reference)
"""Distributed Trainium2 attention kernel (8 NeuronCores).

Strategy: tensor-parallel over heads for QKV projection + attention
(4 query heads + their 1 shared KV head per core, identical causal loop
structure on every core), then an AllToAll switches to row-sharding so
each core computes the output projection for its 512 rows with the full
wo. Host reassembles rows. All matmuls run in bf16 with fp32 PSUM
accumulation; softmax runs unnormalized with the normalization folded in
after the PV matmul (per-head row sums via a ones-matmul).

RoPE is applied in row-major layout via a host-side even/odd column
permutation of wq/wk (rotation becomes contiguous half-block arithmetic),
then q/k are transposed to [head_dim, rows] on the TensorEngine for the
attention matmuls.
"""

import numpy as np
import ml_dtypes

import concourse.bass as bass
import concourse.mybir as mybir
import concourse.tile as tile
from concourse import bacc
from concourse import bass_utils

B, S, D = 2, 2048, 4096
H, HKV, HD = 32, 8, 128
HD2 = HD // 2
NC = 8
HL = H // NC            # 4 local q heads per core
BS = B * S              # 4096 global rows
R = BS // NC            # 512 output rows per core
NRB = BS // 128         # 32 row blocks
NDT = D // 128          # 32 contraction tiles
SCALE = 1.0 / float(np.sqrt(HD))
BF = mybir.dt.bfloat16
F32 = mybir.dt.float32

PROFILE = False         # set by test.py for neuron-profile capture
TMPDIR = None           # set by test.py to keep the trace dir


def _emit(nc, tc, io):
    xT, wqkvT, woT, ccR, ssR, trim, onec, oner, iden, out = io

    with (
        tc.tile_pool(name="ps", bufs=8, space="PSUM") as ps,
        tc.tile_pool(name="wbuf", bufs=1) as wbuf,
        tc.tile_pool(name="qbuf", bufs=1) as qbuf,
        tc.tile_pool(name="kvbuf", bufs=1) as kvbuf,
        tc.tile_pool(name="abuf", bufs=1) as abuf,
        tc.tile_pool(name="cbuf", bufs=1) as cbuf,
        tc.tile_pool(name="xs", bufs=6) as xs,
        tc.tile_pool(name="cs", bufs=3) as cs,
        tc.tile_pool(name="es", bufs=6) as es,
        tc.tile_pool(name="ws", bufs=16) as ws,
        tc.tile_pool(name="ts", bufs=8) as ts,
        tc.tile_pool(name="ans", bufs=4) as ans,
        tc.tile_pool(name="rsp", bufs=4) as rsp,
        tc.tile_pool(name="os", bufs=3) as osp,
        tc.tile_pool(name="dram", bufs=1, space="DRAM") as dram,
    ):
        # ---- constants ----
        trim_sb = cbuf.tile([128, 128], F32, tag="tm")
        nc.sync.dma_start(trim_sb[:], trim[:])
        onec_sb = cbuf.tile([128, 1], BF, tag="oc")
        nc.sync.dma_start(onec_sb[:], onec[:])
        oner_sb = cbuf.tile([1, 128], F32, tag="orr")
        nc.sync.dma_start(oner_sb[:], oner[:])
        iden_sb = cbuf.tile([128, 128], BF, tag="idn")
        nc.sync.dma_start(iden_sb[:], iden[:])

        # resident QKV weights: col = dt*768 + [0:512 q | 512:640 k | 640:768 v]
        w_sb = wbuf.tile([128, NDT * 768], BF, tag="w")
        for dt in range(NDT):
            nc.sync.dma_start(
                w_sb[:, dt * 768: dt * 768 + 768],
                wqkvT[dt * 128: (dt + 1) * 128, :],
            )

        q_sb = qbuf.tile([128, HL * BS], BF, tag="q")     # col = h*4096 + row
        kT_sb = kvbuf.tile([128, BS], BF, tag="k")        # col = row
        v_sb = kvbuf.tile([128, BS], BF, tag="v")         # col = rb*128 + hd

        a2a_in = dram.tile([BS, R], BF, name="a2a_in")
        a2a_out = dram.tile([BS, R], BF, name="a2a_out")

        # ---- phase B: QKV projection + RoPE + transposes ----
        # The rope+transpose tail of row block rb is emitted one iteration
        # late, behind rb+1's matmuls, so the PE queue never stalls on the
        # DVE rope chain.
        def b_rope_tail_q(rb, ps_q):
            cct = cs.tile([128, 256], BF, tag="cc")
            nc.sync.dma_start(cct[:], ccR[:, rb * 256: (rb + 1) * 256])
            sst = cs.tile([128, 256], BF, tag="ss")
            nc.sync.dma_start(sst[:], ssR[:, rb * 256: (rb + 1) * 256])

            # q rotation, all 4 heads at once via strided APs
            qe = ps_q[:].rearrange("p (h d) -> p h d", d=128)[:, :, 0:HD2]
            qo = ps_q[:].rearrange("p (h d) -> p h d", d=128)[:, :, HD2:HD]
            t1 = ts.tile([128, 256], BF, tag="t")
            t2 = ts.tile([128, 256], BF, tag="t")
            t3 = ts.tile([128, 256], BF, tag="t")
            t4 = ts.tile([128, 256], BF, tag="t")
            nc.vector.tensor_mul(t1[:], qe, cct[:])
            nc.vector.tensor_mul(t2[:], qo, sst[:])
            nc.vector.tensor_mul(t3[:], qe, sst[:])
            nc.vector.tensor_mul(t4[:], qo, cct[:])
            qrot = ts.tile([128, 512], BF, tag="qr")
            qre = qrot[:].rearrange("p (h d) -> p h d", d=128)[:, :, 0:HD2]
            qro = qrot[:].rearrange("p (h d) -> p h d", d=128)[:, :, HD2:HD]
            nc.vector.tensor_sub(qre, t1[:], t2[:])
            nc.vector.tensor_add(qro, t3[:], t4[:])
            return (qrot,)

        def b_transpose_tail_q(rb, qrot):
            # transpose q (4 heads, packed into one psum bank)
            ps_tq = ps.tile([128, 512], BF, tag="b")
            for h in range(HL):
                nc.tensor.transpose(
                    ps_tq[:, h * 128: (h + 1) * 128],
                    qrot[:, h * 128: (h + 1) * 128],
                    iden_sb[:],
                )
            q_dst = (
                q_sb[:]
                .rearrange("p (h r) -> p h r", h=HL)
                [:, :, rb * 128: (rb + 1) * 128]
            )
            nc.vector.tensor_copy(
                q_dst, ps_tq[:].rearrange("p (h r) -> p h r", h=HL)
            )

        def b_rope_tail_kv(rb, ps_kv):
            cct = cs.tile([128, 256], BF, tag="cc")
            nc.sync.dma_start(cct[:], ccR[:, rb * 256: (rb + 1) * 256])
            sst = cs.tile([128, 256], BF, tag="ss")
            nc.sync.dma_start(sst[:], ssR[:, rb * 256: (rb + 1) * 256])

            # k rotation (one head)
            ke = ps_kv[:, 0:HD2]
            ko = ps_kv[:, HD2:HD]
            u1 = ts.tile([128, 64], BF, tag="u")
            u2 = ts.tile([128, 64], BF, tag="u")
            u3 = ts.tile([128, 64], BF, tag="u")
            u4 = ts.tile([128, 64], BF, tag="u")
            nc.vector.tensor_mul(u1[:], ke, cct[:, 0:HD2])
            nc.vector.tensor_mul(u2[:], ko, sst[:, 0:HD2])
            nc.vector.tensor_mul(u3[:], ke, sst[:, 0:HD2])
            nc.vector.tensor_mul(u4[:], ko, cct[:, 0:HD2])
            krot = ts.tile([128, 128], BF, tag="kr")
            nc.vector.tensor_sub(krot[:, 0:HD2], u1[:], u2[:])
            nc.vector.tensor_add(krot[:, HD2:HD], u3[:], u4[:])

            # v: plain copy to row-major storage
            nc.scalar.activation(
                v_sb[:, rb * 128: (rb + 1) * 128], ps_kv[:, 128:256],
                mybir.ActivationFunctionType.Copy,
            )
            return (krot,)

        def b_transpose_tail_kv(rb, krot):
            ps_tk = ps.tile([128, 128], BF, tag="b")
            nc.tensor.transpose(ps_tk[:], krot[:], iden_sb[:])
            nc.vector.tensor_copy(kT_sb[:, rb * 128: (rb + 1) * 128], ps_tk[:])

        # Pass 1: q projection only (N=512 matmuls, weight loads fully
        # pipelined). Pass 2: k|v projection (N=256). Separating the two
        # avoids the second, serialized LDWEIGHTS of the shared stationary.
        pending = None
        rot = None
        for rb in range(NRB):
            ps_q = ps.tile([128, 512], F32, tag="b")      # [rows, 4 q heads]
            for dt in range(NDT):
                xt = xs.tile([128, 128], BF, tag="x")
                eng = (nc.gpsimd, nc.scalar, nc.sync)[dt % 3]
                eng.dma_start(
                    xt[:], xT[dt * 128: (dt + 1) * 128, rb * 128: (rb + 1) * 128]
                )
                nc.tensor.matmul(
                    ps_q[:], xt[:], w_sb[:, dt * 768: dt * 768 + 512],
                    start=dt == 0, stop=dt == NDT - 1,
                )
                if dt == 2 and pending is not None:
                    rot = (pending[0],) + b_rope_tail_q(*pending)
                    pending = None
                if dt == 12 and rot is not None:
                    b_transpose_tail_q(*rot)
                    rot = None
            pending = (rb, ps_q)
        rot = (pending[0],) + b_rope_tail_q(*pending)
        b_transpose_tail_q(*rot)

        pending = None
        rot = None
        for rb in range(NRB):
            ps_kv = ps.tile([128, 256], F32, tag="b")     # [rows, k|v]
            for dt in range(NDT):
                xt = xs.tile([128, 128], BF, tag="x")
                eng = (nc.gpsimd, nc.scalar, nc.sync)[dt % 3]
                eng.dma_start(
                    xt[:], xT[dt * 128: (dt + 1) * 128, rb * 128: (rb + 1) * 128]
                )
                nc.tensor.matmul(
                    ps_kv[:], xt[:], w_sb[:, dt * 768 + 512: dt * 768 + 768],
                    start=dt == 0, stop=dt == NDT - 1,
                )
                if dt == 2 and pending is not None:
                    rot = (pending[0],) + b_rope_tail_kv(*pending)
                    pending = None
                if dt == 12 and rot is not None:
                    b_transpose_tail_kv(*rot)
                    rot = None
            pending = (rb, ps_kv)
        rot = (pending[0],) + b_rope_tail_kv(*pending)
        b_transpose_tail_kv(*rot)

        # ---- phase C: causal attention, paired interleaved chains ----
        # Each (b, h, ci) is an independent chain; two chains are emitted
        # interleaved so one chain's exp latency hides under the other's
        # matmuls. Pairing ci=0 with ci=3 (and 1 with 2) balances lengths.
        def attn_chain(b, h, ci):
            qbase = h * BS + b * S
            ps_attn = ps.tile([128, 512], F32, tag="b", name=f"pa{b}{h}{ci}")
            ps_rs = ps.tile([1, 512], F32, tag="b", name=f"pr{b}{h}{ci}")
            jmax = 4 * ci + 3

            def qspan(j):
                q0 = max(j * 128, 512 * ci)
                return q0, 512 * ci + 512 - q0

            def scores(j):
                q0, w = qspan(j)
                kcol = (b * 16 + j) * 128
                ps_s = ps.tile([128, 512], F32, tag="b", name=f"s{j}")
                nc.tensor.matmul(
                    ps_s[:, 0:w],
                    kT_sb[:, kcol: kcol + 128],
                    q_sb[:, qbase + q0: qbase + q0 + w],
                    start=True, stop=True,
                )
                if j // 4 == ci:
                    nc.vector.tensor_add(
                        ps_s[:, 0:128], ps_s[:, 0:128], trim_sb[:]
                    )
                et = es.tile([128, 512], BF, tag="e", name=f"e{j}")
                nc.scalar.activation(
                    et[:, 0:w], ps_s[:, 0:w],
                    mybir.ActivationFunctionType.Exp, scale=SCALE,
                )
                return et

            def pv(j, et):
                q0, w = qspan(j)
                off = q0 - 512 * ci
                kcol = (b * 16 + j) * 128
                nc.tensor.matmul(
                    ps_attn[:, off: off + w],
                    v_sb[:, kcol: kcol + 128],
                    et[:, 0:w],
                    start=(j == 0), stop=(j == jmax),
                )
                nc.tensor.matmul(
                    ps_rs[:, off: off + w],
                    onec_sb[:],
                    et[:, 0:w],
                    start=(j == 0), stop=(j == jmax),
                )

            prev = None
            for j in range(jmax + 1):
                et = scores(j)
                if prev is not None:
                    pv(prev[0], prev[1])
                prev = (j, et)
                yield
            pv(prev[0], prev[1])
            rc = rsp.tile([1, 512], F32, tag="rc")
            nc.vector.reciprocal(rc[:], ps_rs[:])
            bc_sb = rsp.tile([128, 512], F32, tag="bcs")
            nc.gpsimd.partition_broadcast(bc_sb[:], rc[:])
            an = ans.tile([128, 512], BF, tag="an")
            nc.vector.tensor_mul(an[:], ps_attn[:], bc_sb[:])
            blk = 512 * (b * 4 + ci) + 128 * h
            nc.sync.dma_start(a2a_in[blk: blk + 128, :], an[:])
            yield

        # Continuous worklist: always two chains in flight, staggered phases.
        todo = []
        for b in range(B):
            for h in range(HL):
                todo += [(b, h, 0), (b, h, 3), (b, h, 1), (b, h, 2)]
        todo.reverse()
        active = [attn_chain(*todo.pop()), attn_chain(*todo.pop())]
        while active:
            for g in list(active):
                if next(g, StopIteration) is StopIteration:
                    active.remove(g)
                    if todo:
                        active.append(attn_chain(*todo.pop()))

        nc.gpsimd.collective_compute(
            "AllToAll",
            mybir.AluOpType.bypass,
            replica_groups=[list(range(NC))],
            ins=[a2a_in.opt()],
            outs=[a2a_out.opt()],
        )

        # ---- phase D: output projection for this core's 512 rows ----
        at_sb = abuf.tile([128, 32 * 512], BF, tag="at")  # col = ht*512 + row
        for ht in range(32):
            nc.sync.dma_start(
                at_sb[:, ht * 512: (ht + 1) * 512],
                a2a_out[ht * 128: (ht + 1) * 128, :],
            )
        for cg in range(8):
            po = [
                ps.tile([128, 512], F32, tag="b", name=f"po{cg}_{i}")
                for i in range(4)
            ]
            for ht in range(32):
                wt = ws.tile([128, 512], BF, tag="wo")
                weng = (nc.sync, nc.scalar, nc.gpsimd)[ht % 3]
                weng.dma_start(
                    wt[:], woT[ht * 128: (ht + 1) * 128, cg * 512: (cg + 1) * 512]
                )
                for rt in range(4):
                    nc.tensor.matmul(
                        po[rt][:],
                        at_sb[:, ht * 512 + rt * 128: ht * 512 + (rt + 1) * 128],
                        wt[:],
                        start=(ht == 0), stop=(ht == 31),
                    )
            for rt in range(4):
                ot = osp.tile([128, 512], F32, tag="o")
                nc.vector.tensor_copy(ot[:], po[rt][:])
                nc.sync.dma_start(
                    out[rt * 128: (rt + 1) * 128, cg * 512: (cg + 1) * 512], ot[:]
                )


_LDW_PATCHED = False


def _patch_ldw_opt():
    """Enable walrus's redundant-LDWEIGHTS elision (off by default in
    concourse's compile flags; our phase-B matmul pairs share the same
    stationary operand back to back)."""
    global _LDW_PATCHED
    if _LDW_PATCHED:
        return
    _LDW_PATCHED = True
    real_run = bass_utils.run_command

    def run_hook(argv, **kw):
        argv = [
            a.replace("--enable-ldw-opt=false", "--enable-ldw-opt=true")
            if isinstance(a, str) else a
            for a in argv
        ]
        return real_run(argv, **kw)

    bass_utils.run_command = run_hook


def _build():
    nc = bacc.Bacc("TRN2", target_bir_lowering=False, debug=False, num_devices=NC)
    xT = nc.dram_tensor("xT", [D, BS], BF, kind="ExternalInput")
    wqkvT = nc.dram_tensor("wqkvT", [D, 768], BF, kind="ExternalInput")
    woT = nc.dram_tensor("woT", [D, D], BF, kind="ExternalInput")
    ccR = nc.dram_tensor("ccR", [128, NRB * 256], BF, kind="ExternalInput")
    ssR = nc.dram_tensor("ssR", [128, NRB * 256], BF, kind="ExternalInput")
    trim = nc.dram_tensor("trim", [128, 128], F32, kind="ExternalInput")
    onec = nc.dram_tensor("onec", [128, 1], BF, kind="ExternalInput")
    oner = nc.dram_tensor("oner", [1, 128], F32, kind="ExternalInput")
    iden = nc.dram_tensor("iden", [128, 128], BF, kind="ExternalInput")
    out = nc.dram_tensor("out", [R, D], F32, kind="ExternalOutput")
    with tile.TileContext(nc) as tc:
        _emit(nc, tc, (xT, wqkvT, woT, ccR, ssR, trim, onec, oner, iden, out))
    nc.compile()
    return nc


_NC = None


def kernel(x, wq, wk, wv, wo, freqs_cos, freqs_sin, mask, start_pos):
    global _NC
    if _NC is None:
        _NC = _build()
    nc = _NC
    bf = ml_dtypes.bfloat16

    x = np.asarray(x, dtype=np.float32)
    xT = np.ascontiguousarray(x.reshape(BS, D).T).astype(bf)

    perm = np.concatenate([np.arange(0, HD, 2), np.arange(1, HD, 2)])
    wqTp = np.asarray(wq, np.float32).T.reshape(D, H, HD)[:, :, perm]
    wkTp = np.asarray(wk, np.float32).T.reshape(D, HKV, HD)[:, :, perm]
    wvT = np.asarray(wv, np.float32).T.reshape(D, HKV, HD)
    woT = np.ascontiguousarray(np.asarray(wo, np.float32).T).astype(bf)

    fc = np.asarray(freqs_cos, np.float32)
    fs = np.asarray(freqs_sin, np.float32)
    # row-major RoPE tables per row block, replicated x4 along free axis
    pos = (np.arange(BS) % S).reshape(NRB, 128)
    ccR = np.tile(fc[pos], (1, 1, 4)).transpose(1, 0, 2).reshape(128, NRB * 256)
    ssR = np.tile(fs[pos], (1, 1, 4)).transpose(1, 0, 2).reshape(128, NRB * 256)
    ccR = np.ascontiguousarray(ccR).astype(bf)
    ssR = np.ascontiguousarray(ssR).astype(bf)

    trim = np.where(
        np.arange(128)[:, None] > np.arange(128)[None, :], -1e30, 0.0
    ).astype(np.float32)
    onec = np.ones((128, 1), dtype=bf)
    oner = np.ones((1, 128), dtype=np.float32)
    iden = np.eye(128, dtype=bf)

    in_maps = []
    for c in range(NC):
        wqkv = np.concatenate(
            [
                wqTp[:, 4 * c: 4 * c + 4].reshape(D, 512),
                wkTp[:, c],
                wvT[:, c],
            ],
            axis=1,
        ).astype(bf)
        in_maps.append(
            {
                "xT": xT,
                "wqkvT": np.ascontiguousarray(wqkv),
                "woT": woT,
                "ccR": ccR,
                "ssR": ssR,
                "trim": trim,
                "onec": onec,
                "oner": oner,
                "iden": iden,
            }
        )

    res = bass_utils.run_bass_kernel_spmd(
        nc, in_maps, core_ids=list(range(NC)), trace=PROFILE, tmpdir=TMPDIR
    )
    if PROFILE:
        print(f"HW exec time: {res.exec_time_ns} ns")
        if res.instructions_and_trace is not None:
            print(f"trace: {res.instructions_and_trace[1]}")

    out_full = np.empty((BS, D), dtype=np.float32)
    for c in range(NC):
        out_full[R * c: R * (c + 1)] = res.results[c]["out"]
    return out_full.reshape(B, S, D)


# revision 31
# speedup vs baseline: 1.4010x; 1.4010x over previous
"""Distributed Trainium2 attention kernel (8 NeuronCores).

Strategy: tensor-parallel over heads for QKV projection + attention
(4 query heads + their 1 shared KV head per core, identical causal loop
structure on every core), then an AllToAll switches to row-sharding so
each core computes the output projection for its 512 rows with the full
wo. Host reassembles rows. All matmuls run in bf16 with fp32 PSUM
accumulation; softmax runs unnormalized with the normalization folded in
after the PV matmul (per-head row sums via a ones-matmul).

RoPE is applied in row-major layout via a host-side even/odd column
permutation of wq/wk (rotation becomes contiguous half-block arithmetic),
then q/k are transposed to [head_dim, rows] on the TensorEngine for the
attention matmuls.
"""

import numpy as np
import ml_dtypes

import concourse.bass as bass
import concourse.mybir as mybir
import concourse.tile as tile
from concourse import bacc
from concourse import bass_utils

B, S, D = 2, 2048, 4096
H, HKV, HD = 32, 8, 128
HD2 = HD // 2
NC = 8
HL = H // NC            # 4 local q heads per core
BS = B * S              # 4096 global rows
R = BS // NC            # 512 output rows per core
NRB = BS // 128         # 32 row blocks
NDT = D // 128          # 32 contraction tiles
SCALE = 1.0 / float(np.sqrt(HD))
BF = mybir.dt.bfloat16
F32 = mybir.dt.float32

PROFILE = False         # set by test.py for neuron-profile capture
TMPDIR = None           # set by test.py to keep the trace dir


def _emit(nc, tc, io):
    xT, wqkvT, woT, ccR, ssR, trim, onec, oner, iden, out = io

    with (
        tc.tile_pool(name="ps", bufs=8, space="PSUM") as ps,
        tc.tile_pool(name="wbuf", bufs=1) as wbuf,
        tc.tile_pool(name="qbuf", bufs=1) as qbuf,
        tc.tile_pool(name="kvbuf", bufs=1) as kvbuf,
        tc.tile_pool(name="abuf", bufs=1) as abuf,
        tc.tile_pool(name="cbuf", bufs=1) as cbuf,
        tc.tile_pool(name="xs", bufs=6) as xs,
        tc.tile_pool(name="cs", bufs=3) as cs,
        tc.tile_pool(name="es", bufs=6) as es,
        tc.tile_pool(name="ws", bufs=16) as ws,
        tc.tile_pool(name="ts", bufs=8) as ts,
        tc.tile_pool(name="ans", bufs=4) as ans,
        tc.tile_pool(name="rsp", bufs=4) as rsp,
        tc.tile_pool(name="os", bufs=3) as osp,
        tc.tile_pool(name="dram", bufs=1, space="DRAM") as dram,
    ):
        # ---- constants ----
        trim_sb = cbuf.tile([128, 128], F32, tag="tm")
        nc.sync.dma_start(trim_sb[:], trim[:])
        onec_sb = cbuf.tile([128, 1], BF, tag="oc")
        nc.sync.dma_start(onec_sb[:], onec[:])
        oner_sb = cbuf.tile([1, 128], F32, tag="orr")
        nc.sync.dma_start(oner_sb[:], oner[:])
        iden_sb = cbuf.tile([128, 128], BF, tag="idn")
        nc.sync.dma_start(iden_sb[:], iden[:])

        # resident QKV weights: col = dt*768 + [0:512 q | 512:640 k | 640:768 v]
        w_sb = wbuf.tile([128, NDT * 768], BF, tag="w")
        for dt in range(NDT):
            nc.sync.dma_start(
                w_sb[:, dt * 768: dt * 768 + 768],
                wqkvT[dt * 128: (dt + 1) * 128, :],
            )

        q_sb = qbuf.tile([128, HL * BS], BF, tag="q")     # col = h*4096 + row
        kT_sb = kvbuf.tile([128, BS], BF, tag="k")        # col = row
        v_sb = kvbuf.tile([128, BS], BF, tag="v")         # col = rb*128 + hd

        # Two half-size AllToAlls: heads {0,1} and heads {2,3} per core.
        # The first fires mid-attention (its writers finish early), hiding
        # its latency; phase D starts accumulating on the first half while
        # the second is still in flight.
        a2a_in1 = dram.tile([BS // 2, R], BF, name="a2a_in1")
        a2a_out1 = dram.tile([BS // 2, R], BF, name="a2a_out1")
        a2a_in2 = dram.tile([BS // 2, R], BF, name="a2a_in2")
        a2a_out2 = dram.tile([BS // 2, R], BF, name="a2a_out2")

        # ---- phase B: QKV projection + RoPE + transposes ----
        # The rope+transpose tail of row block rb is emitted one iteration
        # late, behind rb+1's matmuls, so the PE queue never stalls on the
        # DVE rope chain.
        def b_rope_tail_q(rb, ps_q):
            cct = cs.tile([128, 256], BF, tag="cc")
            nc.sync.dma_start(cct[:], ccR[:, rb * 256: (rb + 1) * 256])
            sst = cs.tile([128, 256], BF, tag="ss")
            nc.sync.dma_start(sst[:], ssR[:, rb * 256: (rb + 1) * 256])

            # q rotation, all 4 heads at once via strided APs
            qe = ps_q[:].rearrange("p (h d) -> p h d", d=128)[:, :, 0:HD2]
            qo = ps_q[:].rearrange("p (h d) -> p h d", d=128)[:, :, HD2:HD]
            t1 = ts.tile([128, 256], BF, tag="t")
            t2 = ts.tile([128, 256], BF, tag="t")
            t3 = ts.tile([128, 256], BF, tag="t")
            t4 = ts.tile([128, 256], BF, tag="t")
            nc.vector.tensor_mul(t1[:], qe, cct[:])
            nc.vector.tensor_mul(t2[:], qo, sst[:])
            nc.vector.tensor_mul(t3[:], qe, sst[:])
            nc.vector.tensor_mul(t4[:], qo, cct[:])
            qrot = ts.tile([128, 512], BF, tag="qr")
            qre = qrot[:].rearrange("p (h d) -> p h d", d=128)[:, :, 0:HD2]
            qro = qrot[:].rearrange("p (h d) -> p h d", d=128)[:, :, HD2:HD]
            nc.vector.tensor_sub(qre, t1[:], t2[:])
            nc.vector.tensor_add(qro, t3[:], t4[:])
            return (qrot,)

        def b_transpose_tail_q(rb, qrot):
            # transpose q (4 heads, packed into one psum bank)
            ps_tq = ps.tile([128, 512], BF, tag="b")
            for h in range(HL):
                nc.tensor.transpose(
                    ps_tq[:, h * 128: (h + 1) * 128],
                    qrot[:, h * 128: (h + 1) * 128],
                    iden_sb[:],
                )
            q_dst = (
                q_sb[:]
                .rearrange("p (h r) -> p h r", h=HL)
                [:, :, rb * 128: (rb + 1) * 128]
            )
            nc.vector.tensor_copy(
                q_dst, ps_tq[:].rearrange("p (h r) -> p h r", h=HL)
            )

        def b_rope_tail_kv(rb, ps_kv):
            cct = cs.tile([128, 256], BF, tag="cc")
            nc.sync.dma_start(cct[:], ccR[:, rb * 256: (rb + 1) * 256])
            sst = cs.tile([128, 256], BF, tag="ss")
            nc.sync.dma_start(sst[:], ssR[:, rb * 256: (rb + 1) * 256])

            # k rotation (one head)
            ke = ps_kv[:, 0:HD2]
            ko = ps_kv[:, HD2:HD]
            u1 = ts.tile([128, 64], BF, tag="u")
            u2 = ts.tile([128, 64], BF, tag="u")
            u3 = ts.tile([128, 64], BF, tag="u")
            u4 = ts.tile([128, 64], BF, tag="u")
            nc.vector.tensor_mul(u1[:], ke, cct[:, 0:HD2])
            nc.vector.tensor_mul(u2[:], ko, sst[:, 0:HD2])
            nc.vector.tensor_mul(u3[:], ke, sst[:, 0:HD2])
            nc.vector.tensor_mul(u4[:], ko, cct[:, 0:HD2])
            krot = ts.tile([128, 128], BF, tag="kr")
            nc.vector.tensor_sub(krot[:, 0:HD2], u1[:], u2[:])
            nc.vector.tensor_add(krot[:, HD2:HD], u3[:], u4[:])

            # v: plain copy to row-major storage
            nc.scalar.activation(
                v_sb[:, rb * 128: (rb + 1) * 128], ps_kv[:, 128:256],
                mybir.ActivationFunctionType.Copy,
            )
            return (krot,)

        def b_transpose_tail_kv(rb, krot):
            ps_tk = ps.tile([128, 128], BF, tag="b")
            nc.tensor.transpose(ps_tk[:], krot[:], iden_sb[:])
            nc.vector.tensor_copy(kT_sb[:, rb * 128: (rb + 1) * 128], ps_tk[:])

        pending = None
        rot = None
        for rb in range(NRB):
            ps_q = ps.tile([128, 512], F32, tag="b")      # [rows, 4 q heads]
            ps_kv = ps.tile([128, 256], F32, tag="b")     # [rows, k|v]
            for dt in range(NDT):
                xt = xs.tile([128, 128], BF, tag="x")
                eng = (nc.gpsimd, nc.scalar, nc.sync)[dt % 3]
                eng.dma_start(
                    xt[:], xT[dt * 128: (dt + 1) * 128, rb * 128: (rb + 1) * 128]
                )
                st, sp = dt == 0, dt == NDT - 1
                nc.tensor.matmul(
                    ps_q[:], xt[:], w_sb[:, dt * 768: dt * 768 + 512],
                    start=st, stop=sp,
                )
                nc.tensor.matmul(
                    ps_kv[:], xt[:], w_sb[:, dt * 768 + 512: dt * 768 + 768],
                    start=st, stop=sp,
                )
                if dt == 2 and pending is not None:
                    rot = (pending[0],) + b_rope_tail_q(pending[0], pending[1]) \
                        + b_rope_tail_kv(pending[0], pending[2])
                    pending = None
                if dt == 12 and rot is not None:
                    b_transpose_tail_q(rot[0], rot[1])
                    b_transpose_tail_kv(rot[0], rot[2])
                    rot = None
            pending = (rb, ps_q, ps_kv)
        rot = (pending[0],) + b_rope_tail_q(pending[0], pending[1]) \
            + b_rope_tail_kv(pending[0], pending[2])
        b_transpose_tail_q(rot[0], rot[1])
        b_transpose_tail_kv(rot[0], rot[2])

        # ---- phase C: causal attention, paired interleaved chains ----
        # Each (b, h, ci) is an independent chain; two chains are emitted
        # interleaved so one chain's exp latency hides under the other's
        # matmuls. Pairing ci=0 with ci=3 (and 1 with 2) balances lengths.
        def attn_chain(b, h, ci):
            qbase = h * BS + b * S
            ps_attn = ps.tile([128, 512], F32, tag="b", name=f"pa{b}{h}{ci}")
            ps_rs = ps.tile([1, 512], F32, tag="b", name=f"pr{b}{h}{ci}")
            jmax = 4 * ci + 3

            def qspan(j):
                q0 = max(j * 128, 512 * ci)
                return q0, 512 * ci + 512 - q0

            def scores(j):
                q0, w = qspan(j)
                kcol = (b * 16 + j) * 128
                ps_s = ps.tile([128, 512], F32, tag="b", name=f"s{j}")
                nc.tensor.matmul(
                    ps_s[:, 0:w],
                    kT_sb[:, kcol: kcol + 128],
                    q_sb[:, qbase + q0: qbase + q0 + w],
                    start=True, stop=True,
                )
                if j // 4 == ci:
                    nc.vector.tensor_add(
                        ps_s[:, 0:128], ps_s[:, 0:128], trim_sb[:]
                    )
                et = es.tile([128, 512], BF, tag="e", name=f"e{j}")
                nc.scalar.activation(
                    et[:, 0:w], ps_s[:, 0:w],
                    mybir.ActivationFunctionType.Exp, scale=SCALE,
                )
                return et

            def pv(j, et):
                q0, w = qspan(j)
                off = q0 - 512 * ci
                kcol = (b * 16 + j) * 128
                nc.tensor.matmul(
                    ps_attn[:, off: off + w],
                    v_sb[:, kcol: kcol + 128],
                    et[:, 0:w],
                    start=(j == 0), stop=(j == jmax),
                )
                nc.tensor.matmul(
                    ps_rs[:, off: off + w],
                    onec_sb[:],
                    et[:, 0:w],
                    start=(j == 0), stop=(j == jmax),
                )

            prev = None
            for j in range(jmax + 1):
                et = scores(j)
                if prev is not None:
                    pv(prev[0], prev[1])
                prev = (j, et)
                yield
            pv(prev[0], prev[1])
            rc = rsp.tile([1, 512], F32, tag="rc")
            nc.vector.reciprocal(rc[:], ps_rs[:])
            bc_sb = rsp.tile([128, 512], F32, tag="bcs")
            nc.gpsimd.partition_broadcast(bc_sb[:], rc[:])
            an = ans.tile([128, 512], BF, tag="an")
            nc.vector.tensor_mul(an[:], ps_attn[:], bc_sb[:])
            dst = a2a_in1 if h < 2 else a2a_in2
            blk = 256 * (b * 4 + ci) + 128 * (h % 2)
            nc.sync.dma_start(dst[blk: blk + 128, :], an[:])
            yield

        # Continuous worklist: always two chains in flight, staggered phases.
        # Heads 0,1 first (both batches) so the first AllToAll's inputs
        # complete halfway through the attention phase.
        todo = []
        for h01 in ((0, 1), (2, 3)):
            for b in range(B):
                for h in h01:
                    todo += [(b, h, 0), (b, h, 3), (b, h, 1), (b, h, 2)]
        todo.reverse()
        active = [attn_chain(*todo.pop()), attn_chain(*todo.pop())]
        while active:
            for g in list(active):
                if next(g, StopIteration) is StopIteration:
                    active.remove(g)
                    if todo:
                        active.append(attn_chain(*todo.pop()))

        nc.gpsimd.collective_compute(
            "AllToAll",
            mybir.AluOpType.bypass,
            replica_groups=[list(range(NC))],
            ins=[a2a_in1.opt()],
            outs=[a2a_out1.opt()],
        )
        nc.gpsimd.collective_compute(
            "AllToAll",
            mybir.AluOpType.bypass,
            replica_groups=[list(range(NC))],
            ins=[a2a_in2.opt()],
            outs=[a2a_out2.opt()],
        )

        # ---- phase D: output projection for this core's 512 rows ----
        # ht order: first the tiles delivered by the first AllToAll
        # (head%4 in {0,1}), so accumulation overlaps the second one.
        ht_order = [4 * i + l for l in (0, 1) for i in range(8)] + \
                   [4 * i + l for l in (2, 3) for i in range(8)]
        at_sb = abuf.tile([128, 32 * 512], BF, tag="at")  # col = ht*512 + row
        for ht in ht_order:
            i, htl = ht // 4, ht % 4
            src = a2a_out1 if htl < 2 else a2a_out2
            srow = (i * 2 + (htl % 2)) * 128
            nc.sync.dma_start(
                at_sb[:, ht * 512: (ht + 1) * 512],
                src[srow: srow + 128, :],
            )
        for cg in range(8):
            po = [
                ps.tile([128, 512], F32, tag="b", name=f"po{cg}_{i}")
                for i in range(4)
            ]
            for n_ht, ht in enumerate(ht_order):
                wt = ws.tile([128, 512], BF, tag="wo")
                weng = (nc.sync, nc.scalar, nc.gpsimd)[ht % 3]
                weng.dma_start(
                    wt[:], woT[ht * 128: (ht + 1) * 128, cg * 512: (cg + 1) * 512]
                )
                for rt in range(4):
                    nc.tensor.matmul(
                        po[rt][:],
                        at_sb[:, ht * 512 + rt * 128: ht * 512 + (rt + 1) * 128],
                        wt[:],
                        start=(n_ht == 0), stop=(n_ht == 31),
                    )
            for rt in range(4):
                ot = osp.tile([128, 512], F32, tag="o")
                nc.vector.tensor_copy(ot[:], po[rt][:])
                nc.sync.dma_start(
                    out[rt * 128: (rt + 1) * 128, cg * 512: (cg + 1) * 512], ot[:]
                )


_LDW_PATCHED = False


def _patch_ldw_opt():
    """Enable walrus's redundant-LDWEIGHTS elision (off by default in
    concourse's compile flags; our phase-B matmul pairs share the same
    stationary operand back to back)."""
    global _LDW_PATCHED
    if _LDW_PATCHED:
        return
    _LDW_PATCHED = True
    real_run = bass_utils.run_command

    def run_hook(argv, **kw):
        argv = [
            a.replace("--enable-ldw-opt=false", "--enable-ldw-opt=true")
            if isinstance(a, str) else a
            for a in argv
        ]
        return real_run(argv, **kw)

    bass_utils.run_command = run_hook


def _build():
    nc = bacc.Bacc("TRN2", target_bir_lowering=False, debug=False, num_devices=NC)
    xT = nc.dram_tensor("xT", [D, BS], BF, kind="ExternalInput")
    wqkvT = nc.dram_tensor("wqkvT", [D, 768], BF, kind="ExternalInput")
    woT = nc.dram_tensor("woT", [D, D], BF, kind="ExternalInput")
    ccR = nc.dram_tensor("ccR", [128, NRB * 256], BF, kind="ExternalInput")
    ssR = nc.dram_tensor("ssR", [128, NRB * 256], BF, kind="ExternalInput")
    trim = nc.dram_tensor("trim", [128, 128], F32, kind="ExternalInput")
    onec = nc.dram_tensor("onec", [128, 1], BF, kind="ExternalInput")
    oner = nc.dram_tensor("oner", [1, 128], F32, kind="ExternalInput")
    iden = nc.dram_tensor("iden", [128, 128], BF, kind="ExternalInput")
    out = nc.dram_tensor("out", [R, D], F32, kind="ExternalOutput")
    with tile.TileContext(nc) as tc:
        _emit(nc, tc, (xT, wqkvT, woT, ccR, ssR, trim, onec, oner, iden, out))
    nc.compile()
    return nc


_NC = None


def kernel(x, wq, wk, wv, wo, freqs_cos, freqs_sin, mask, start_pos):
    global _NC
    if _NC is None:
        _NC = _build()
    nc = _NC
    bf = ml_dtypes.bfloat16

    x = np.asarray(x, dtype=np.float32)
    xT = np.ascontiguousarray(x.reshape(BS, D).T).astype(bf)

    perm = np.concatenate([np.arange(0, HD, 2), np.arange(1, HD, 2)])
    wqTp = np.asarray(wq, np.float32).T.reshape(D, H, HD)[:, :, perm]
    wkTp = np.asarray(wk, np.float32).T.reshape(D, HKV, HD)[:, :, perm]
    wvT = np.asarray(wv, np.float32).T.reshape(D, HKV, HD)
    woT = np.ascontiguousarray(np.asarray(wo, np.float32).T).astype(bf)

    fc = np.asarray(freqs_cos, np.float32)
    fs = np.asarray(freqs_sin, np.float32)
    # row-major RoPE tables per row block, replicated x4 along free axis
    pos = (np.arange(BS) % S).reshape(NRB, 128)
    ccR = np.tile(fc[pos], (1, 1, 4)).transpose(1, 0, 2).reshape(128, NRB * 256)
    ssR = np.tile(fs[pos], (1, 1, 4)).transpose(1, 0, 2).reshape(128, NRB * 256)
    ccR = np.ascontiguousarray(ccR).astype(bf)
    ssR = np.ascontiguousarray(ssR).astype(bf)

    trim = np.where(
        np.arange(128)[:, None] > np.arange(128)[None, :], -1e30, 0.0
    ).astype(np.float32)
    onec = np.ones((128, 1), dtype=bf)
    oner = np.ones((1, 128), dtype=np.float32)
    iden = np.eye(128, dtype=bf)

    in_maps = []
    for c in range(NC):
        wqkv = np.concatenate(
            [
                wqTp[:, 4 * c: 4 * c + 4].reshape(D, 512),
                wkTp[:, c],
                wvT[:, c],
            ],
            axis=1,
        ).astype(bf)
        in_maps.append(
            {
                "xT": xT,
                "wqkvT": np.ascontiguousarray(wqkv),
                "woT": woT,
                "ccR": ccR,
                "ssR": ssR,
                "trim": trim,
                "onec": onec,
                "oner": oner,
                "iden": iden,
            }
        )

    res = bass_utils.run_bass_kernel_spmd(
        nc, in_maps, core_ids=list(range(NC)), trace=PROFILE, tmpdir=TMPDIR
    )
    if PROFILE:
        print(f"HW exec time: {res.exec_time_ns} ns")
        if res.instructions_and_trace is not None:
            print(f"trace: {res.instructions_and_trace[1]}")

    out_full = np.empty((BS, D), dtype=np.float32)
    for c in range(NC):
        out_full[R * c: R * (c + 1)] = res.results[c]["out"]
    return out_full.reshape(B, S, D)


# revision 35
# speedup vs baseline: 1.4074x; 1.0045x over previous
"""Distributed Trainium2 attention kernel (8 NeuronCores).

Strategy: tensor-parallel over heads for QKV projection + attention
(4 query heads + their 1 shared KV head per core, identical causal loop
structure on every core), then an AllToAll switches to row-sharding so
each core computes the output projection for its 512 rows with the full
wo. Host reassembles rows. All matmuls run in bf16 with fp32 PSUM
accumulation; softmax runs unnormalized with the normalization folded in
after the PV matmul (per-head row sums via a ones-matmul).

RoPE is applied in row-major layout via a host-side even/odd column
permutation of wq/wk (rotation becomes contiguous half-block arithmetic),
then q/k are transposed to [head_dim, rows] on the TensorEngine for the
attention matmuls.
"""

import numpy as np
import ml_dtypes

import concourse.bass as bass
import concourse.mybir as mybir
import concourse.tile as tile
from concourse import bacc
from concourse import bass_utils

B, S, D = 2, 2048, 4096
H, HKV, HD = 32, 8, 128
HD2 = HD // 2
NC = 8
HL = H // NC            # 4 local q heads per core
BS = B * S              # 4096 global rows
R = BS // NC            # 512 output rows per core
NRB = BS // 128         # 32 row blocks
NDT = D // 128          # 32 contraction tiles
SCALE = 1.0 / float(np.sqrt(HD))
BF = mybir.dt.bfloat16
F32 = mybir.dt.float32

PROFILE = False         # set by test.py for neuron-profile capture
TMPDIR = None           # set by test.py to keep the trace dir


def _emit(nc, tc, io):
    xT, wqkvT, woT, ccR, ssR, trim, onec, oner, iden, out = io

    with (
        tc.tile_pool(name="ps", bufs=8, space="PSUM") as ps,
        tc.tile_pool(name="wbuf", bufs=1) as wbuf,
        tc.tile_pool(name="qbuf", bufs=1) as qbuf,
        tc.tile_pool(name="kvbuf", bufs=1) as kvbuf,
        tc.tile_pool(name="abuf", bufs=1) as abuf,
        tc.tile_pool(name="cbuf", bufs=1) as cbuf,
        tc.tile_pool(name="xs", bufs=6) as xs,
        tc.tile_pool(name="cs", bufs=3) as cs,
        tc.tile_pool(name="es", bufs=6) as es,
        tc.tile_pool(name="ws", bufs=16) as ws,
        tc.tile_pool(name="ts", bufs=8) as ts,
        tc.tile_pool(name="ans", bufs=4) as ans,
        tc.tile_pool(name="rsp", bufs=4) as rsp,
        tc.tile_pool(name="os", bufs=3) as osp,
        tc.tile_pool(name="dram", bufs=1, space="DRAM") as dram,
    ):
        # ---- constants ----
        trim_sb = cbuf.tile([128, 128], F32, tag="tm")
        nc.sync.dma_start(trim_sb[:], trim[:])
        onec_sb = cbuf.tile([128, 1], BF, tag="oc")
        nc.sync.dma_start(onec_sb[:], onec[:])
        oner_sb = cbuf.tile([1, 128], F32, tag="orr")
        nc.sync.dma_start(oner_sb[:], oner[:])
        iden_sb = cbuf.tile([128, 128], BF, tag="idn")
        nc.sync.dma_start(iden_sb[:], iden[:])

        # resident QKV weights: col = dt*768 + [0:512 q | 512:640 k | 640:768 v]
        w_sb = wbuf.tile([128, NDT * 768], BF, tag="w")
        for dt in range(NDT):
            nc.sync.dma_start(
                w_sb[:, dt * 768: dt * 768 + 768],
                wqkvT[dt * 128: (dt + 1) * 128, :],
            )

        q_sb = qbuf.tile([128, HL * BS], BF, tag="q")     # col = h*4096 + row
        kT_sb = kvbuf.tile([128, BS], BF, tag="k")        # col = row
        v_sb = kvbuf.tile([128, BS], BF, tag="v")         # col = rb*128 + hd

        # Two half-size AllToAlls: heads {0,1} and heads {2,3} per core.
        # The first fires mid-attention (its writers finish early), hiding
        # its latency; phase D starts accumulating on the first half while
        # the second is still in flight.
        a2a_in1 = dram.tile([BS // 2, R], BF, name="a2a_in1")
        a2a_out1 = dram.tile([BS // 2, R], BF, name="a2a_out1")
        a2a_in2 = dram.tile([BS // 2, R], BF, name="a2a_in2")
        a2a_out2 = dram.tile([BS // 2, R], BF, name="a2a_out2")

        # ---- phase B: QKV projection + RoPE + transposes ----
        # The rope+transpose tail of row block rb is emitted one iteration
        # late, behind rb+1's matmuls, so the PE queue never stalls on the
        # DVE rope chain.
        def b_rope_tail_q(rb, ps_q):
            cct = cs.tile([128, 256], BF, tag="cc")
            nc.sync.dma_start(cct[:], ccR[:, rb * 256: (rb + 1) * 256])
            sst = cs.tile([128, 256], BF, tag="ss")
            nc.sync.dma_start(sst[:], ssR[:, rb * 256: (rb + 1) * 256])

            # q rotation, all 4 heads at once via strided APs
            qe = ps_q[:].rearrange("p (h d) -> p h d", d=128)[:, :, 0:HD2]
            qo = ps_q[:].rearrange("p (h d) -> p h d", d=128)[:, :, HD2:HD]
            t1 = ts.tile([128, 256], BF, tag="t")
            t2 = ts.tile([128, 256], BF, tag="t")
            t3 = ts.tile([128, 256], BF, tag="t")
            t4 = ts.tile([128, 256], BF, tag="t")
            nc.vector.tensor_mul(t1[:], qe, cct[:])
            nc.vector.tensor_mul(t2[:], qo, sst[:])
            nc.vector.tensor_mul(t3[:], qe, sst[:])
            nc.vector.tensor_mul(t4[:], qo, cct[:])
            qrot = ts.tile([128, 512], BF, tag="qr")
            qre = qrot[:].rearrange("p (h d) -> p h d", d=128)[:, :, 0:HD2]
            qro = qrot[:].rearrange("p (h d) -> p h d", d=128)[:, :, HD2:HD]
            nc.vector.tensor_sub(qre, t1[:], t2[:])
            nc.vector.tensor_add(qro, t3[:], t4[:])
            return (qrot,)

        def b_transpose_tail_q(rb, qrot):
            # transpose q (4 heads, packed into one psum bank)
            ps_tq = ps.tile([128, 512], BF, tag="b")
            for h in range(HL):
                nc.tensor.transpose(
                    ps_tq[:, h * 128: (h + 1) * 128],
                    qrot[:, h * 128: (h + 1) * 128],
                    iden_sb[:],
                )
            q_dst = (
                q_sb[:]
                .rearrange("p (h r) -> p h r", h=HL)
                [:, :, rb * 128: (rb + 1) * 128]
            )
            nc.vector.tensor_copy(
                q_dst, ps_tq[:].rearrange("p (h r) -> p h r", h=HL)
            )

        def b_rope_tail_kv(rb, ps_kv):
            cct = cs.tile([128, 256], BF, tag="cc")
            nc.sync.dma_start(cct[:], ccR[:, rb * 256: (rb + 1) * 256])
            sst = cs.tile([128, 256], BF, tag="ss")
            nc.sync.dma_start(sst[:], ssR[:, rb * 256: (rb + 1) * 256])

            # k rotation (one head)
            ke = ps_kv[:, 0:HD2]
            ko = ps_kv[:, HD2:HD]
            u1 = ts.tile([128, 64], BF, tag="u")
            u2 = ts.tile([128, 64], BF, tag="u")
            u3 = ts.tile([128, 64], BF, tag="u")
            u4 = ts.tile([128, 64], BF, tag="u")
            nc.vector.tensor_mul(u1[:], ke, cct[:, 0:HD2])
            nc.vector.tensor_mul(u2[:], ko, sst[:, 0:HD2])
            nc.vector.tensor_mul(u3[:], ke, sst[:, 0:HD2])
            nc.vector.tensor_mul(u4[:], ko, cct[:, 0:HD2])
            krot = ts.tile([128, 128], BF, tag="kr")
            nc.vector.tensor_sub(krot[:, 0:HD2], u1[:], u2[:])
            nc.vector.tensor_add(krot[:, HD2:HD], u3[:], u4[:])

            # v: plain copy to row-major storage
            nc.scalar.activation(
                v_sb[:, rb * 128: (rb + 1) * 128], ps_kv[:, 128:256],
                mybir.ActivationFunctionType.Copy,
            )
            return (krot,)

        def b_transpose_tail_kv(rb, krot):
            ps_tk = ps.tile([128, 128], BF, tag="b")
            nc.tensor.transpose(ps_tk[:], krot[:], iden_sb[:])
            nc.vector.tensor_copy(kT_sb[:, rb * 128: (rb + 1) * 128], ps_tk[:])

        pending = None
        rot = None
        for rb in range(NRB):
            ps_q = ps.tile([128, 512], F32, tag="b")      # [rows, 4 q heads]
            ps_kv = ps.tile([128, 256], F32, tag="b")     # [rows, k|v]
            for dt in range(NDT):
                xt = xs.tile([128, 128], BF, tag="x")
                eng = (nc.gpsimd, nc.scalar, nc.sync)[dt % 3]
                eng.dma_start(
                    xt[:], xT[dt * 128: (dt + 1) * 128, rb * 128: (rb + 1) * 128]
                )
                st, sp = dt == 0, dt == NDT - 1
                nc.tensor.matmul(
                    ps_q[:], xt[:], w_sb[:, dt * 768: dt * 768 + 512],
                    start=st, stop=sp,
                )
                nc.tensor.matmul(
                    ps_kv[:], xt[:], w_sb[:, dt * 768 + 512: dt * 768 + 768],
                    start=st, stop=sp,
                )
                if dt == 2 and pending is not None:
                    rot = (pending[0],) + b_rope_tail_q(pending[0], pending[1]) \
                        + b_rope_tail_kv(pending[0], pending[2])
                    pending = None
                if dt == 12 and rot is not None:
                    b_transpose_tail_q(rot[0], rot[1])
                    b_transpose_tail_kv(rot[0], rot[2])
                    rot = None
            pending = (rb, ps_q, ps_kv)
        rot = (pending[0],) + b_rope_tail_q(pending[0], pending[1]) \
            + b_rope_tail_kv(pending[0], pending[2])
        b_transpose_tail_q(rot[0], rot[1])
        b_transpose_tail_kv(rot[0], rot[2])

        # ---- phase C: causal attention, paired interleaved chains ----
        # Each (b, h, ci) is an independent chain; two chains are emitted
        # interleaved so one chain's exp latency hides under the other's
        # matmuls. Pairing ci=0 with ci=3 (and 1 with 2) balances lengths.
        def attn_chain(b, h, ci):
            qbase = h * BS + b * S
            ps_attn = ps.tile([128, 512], F32, tag="b", name=f"pa{b}{h}{ci}")
            ps_rs = ps.tile([1, 512], F32, tag="b", name=f"pr{b}{h}{ci}")
            jmax = 4 * ci + 3

            def qspan(j):
                q0 = max(j * 128, 512 * ci)
                return q0, 512 * ci + 512 - q0

            def scores(j):
                q0, w = qspan(j)
                kcol = (b * 16 + j) * 128
                ps_s = ps.tile([128, 512], F32, tag="b", name=f"s{j}")
                nc.tensor.matmul(
                    ps_s[:, 0:w],
                    kT_sb[:, kcol: kcol + 128],
                    q_sb[:, qbase + q0: qbase + q0 + w],
                    start=True, stop=True,
                )
                if j // 4 == ci:
                    nc.vector.tensor_add(
                        ps_s[:, 0:128], ps_s[:, 0:128], trim_sb[:]
                    )
                et = es.tile([128, 512], BF, tag="e", name=f"e{j}")
                nc.scalar.activation(
                    et[:, 0:w], ps_s[:, 0:w],
                    mybir.ActivationFunctionType.Exp, scale=SCALE,
                )
                return et

            def pv(j, et):
                q0, w = qspan(j)
                off = q0 - 512 * ci
                kcol = (b * 16 + j) * 128
                nc.tensor.matmul(
                    ps_attn[:, off: off + w],
                    v_sb[:, kcol: kcol + 128],
                    et[:, 0:w],
                    start=(j == 0), stop=(j == jmax),
                )
                nc.tensor.matmul(
                    ps_rs[:, off: off + w],
                    onec_sb[:],
                    et[:, 0:w],
                    start=(j == 0), stop=(j == jmax),
                )

            prev = None
            for j in range(jmax + 1):
                et = scores(j)
                if prev is not None:
                    pv(prev[0], prev[1])
                prev = (j, et)
                yield
            pv(prev[0], prev[1])
            rc = rsp.tile([1, 512], F32, tag="rc")
            nc.vector.reciprocal(rc[:], ps_rs[:])
            bc_sb = rsp.tile([128, 512], F32, tag="bcs")
            nc.gpsimd.partition_broadcast(bc_sb[:], rc[:])
            an = ans.tile([128, 512], BF, tag="an")
            nc.vector.tensor_mul(an[:], ps_attn[:], bc_sb[:])
            dst = a2a_in1 if h < 2 else a2a_in2
            blk = 256 * (b * 4 + ci) + 128 * (h % 2)
            nc.sync.dma_start(dst[blk: blk + 128, :], an[:])
            yield

        # Continuous worklist: always two chains in flight, staggered phases.
        # Heads 0,1 first (both batches) so the first AllToAll's inputs
        # complete halfway through the attention phase.
        todo = []
        for h01 in ((0, 1), (2, 3)):
            for b in range(B):
                for h in h01:
                    todo += [(b, h, 0), (b, h, 3), (b, h, 1), (b, h, 2)]
        todo.reverse()
        active = [attn_chain(*todo.pop()), attn_chain(*todo.pop())]
        while active:
            for g in list(active):
                if next(g, StopIteration) is StopIteration:
                    active.remove(g)
                    if todo:
                        active.append(attn_chain(*todo.pop()))

        nc.gpsimd.collective_compute(
            "AllToAll",
            mybir.AluOpType.bypass,
            replica_groups=[list(range(NC))],
            ins=[a2a_in1.opt()],
            outs=[a2a_out1.opt()],
        )
        nc.gpsimd.collective_compute(
            "AllToAll",
            mybir.AluOpType.bypass,
            replica_groups=[list(range(NC))],
            ins=[a2a_in2.opt()],
            outs=[a2a_out2.opt()],
        )

        # ---- phase D: output projection for this core's 512 rows ----
        # ht order: first the tiles delivered by the first AllToAll
        # (head%4 in {0,1}), so accumulation overlaps the second one.
        ht_order = [4 * i + l for l in (0, 1) for i in range(8)] + \
                   [4 * i + l for l in (2, 3) for i in range(8)]
        at_sb = abuf.tile([128, 32 * 512], BF, tag="at")  # col = ht*512 + row
        for ht in ht_order:
            i, htl = ht // 4, ht % 4
            src = a2a_out1 if htl < 2 else a2a_out2
            srow = (i * 2 + (htl % 2)) * 128
            nc.sync.dma_start(
                at_sb[:, ht * 512: (ht + 1) * 512],
                src[srow: srow + 128, :],
            )
        for cg in range(8):
            po = [
                ps.tile([128, 512], F32, tag="b", name=f"po{cg}_{i}")
                for i in range(4)
            ]
            for n_ht, ht in enumerate(ht_order):
                wt = ws.tile([128, 512], BF, tag="wo")
                weng = (nc.sync, nc.scalar, nc.gpsimd)[ht % 3]
                weng.dma_start(
                    wt[:], woT[ht * 128: (ht + 1) * 128, cg * 512: (cg + 1) * 512]
                )
                for rt in range(4):
                    nc.tensor.matmul(
                        po[rt][:],
                        at_sb[:, ht * 512 + rt * 128: ht * 512 + (rt + 1) * 128],
                        wt[:],
                        start=(n_ht == 0), stop=(n_ht == 31),
                    )
            for rt in range(4):
                ot = osp.tile([128, 512], F32, tag="o")
                nc.vector.tensor_copy(ot[:], po[rt][:])
                nc.sync.dma_start(
                    out[rt * 128: (rt + 1) * 128, cg * 512: (cg + 1) * 512], ot[:]
                )


_LDW_PATCHED = False


def _patch_ldw_opt():
    """Enable walrus's redundant-LDWEIGHTS elision (off by default in
    concourse's compile flags; our phase-B matmul pairs share the same
    stationary operand back to back)."""
    global _LDW_PATCHED
    if _LDW_PATCHED:
        return
    _LDW_PATCHED = True
    real_run = bass_utils.run_command

    def run_hook(argv, **kw):
        argv = [
            a.replace("--enable-ldw-opt=false", "--enable-ldw-opt=true")
            if isinstance(a, str) else a
            for a in argv
        ]
        return real_run(argv, **kw)

    bass_utils.run_command = run_hook


def _build():
    nc = bacc.Bacc("TRN2", target_bir_lowering=False, debug=False, num_devices=NC)
    xT = nc.dram_tensor("xT", [D, BS], BF, kind="ExternalInput")
    wqkvT = nc.dram_tensor("wqkvT", [D, 768], BF, kind="ExternalInput")
    woT = nc.dram_tensor("woT", [D, D], BF, kind="ExternalInput")
    ccR = nc.dram_tensor("ccR", [128, NRB * 256], BF, kind="ExternalInput")
    ssR = nc.dram_tensor("ssR", [128, NRB * 256], BF, kind="ExternalInput")
    trim = nc.dram_tensor("trim", [128, 128], F32, kind="ExternalInput")
    onec = nc.dram_tensor("onec", [128, 1], BF, kind="ExternalInput")
    oner = nc.dram_tensor("oner", [1, 128], F32, kind="ExternalInput")
    iden = nc.dram_tensor("iden", [128, 128], BF, kind="ExternalInput")
    out = nc.dram_tensor("out", [R, D], F32, kind="ExternalOutput")
    with tile.TileContext(nc) as tc:
        _emit(nc, tc, (xT, wqkvT, woT, ccR, ssR, trim, onec, oner, iden, out))
    nc.compile()
    return nc


_NC = None


def kernel(x, wq, wk, wv, wo, freqs_cos, freqs_sin, mask, start_pos):
    global _NC
    if _NC is None:
        _NC = _build()
    nc = _NC
    bf = ml_dtypes.bfloat16

    x = np.asarray(x, dtype=np.float32)
    xT = np.ascontiguousarray(x.reshape(BS, D).T).astype(bf)

    perm = np.concatenate([np.arange(0, HD, 2), np.arange(1, HD, 2)])
    wqTp = np.asarray(wq, np.float32).T.reshape(D, H, HD)[:, :, perm]
    wkTp = np.asarray(wk, np.float32).T.reshape(D, HKV, HD)[:, :, perm]
    wvT = np.asarray(wv, np.float32).T.reshape(D, HKV, HD)
    woT = np.ascontiguousarray(np.asarray(wo, np.float32).T).astype(bf)

    fc = np.asarray(freqs_cos, np.float32)
    fs = np.asarray(freqs_sin, np.float32)
    # row-major RoPE tables per row block, replicated x4 along free axis
    pos = (np.arange(BS) % S).reshape(NRB, 128)
    ccR = np.tile(fc[pos], (1, 1, 4)).transpose(1, 0, 2).reshape(128, NRB * 256)
    ssR = np.tile(fs[pos], (1, 1, 4)).transpose(1, 0, 2).reshape(128, NRB * 256)
    ccR = np.ascontiguousarray(ccR).astype(bf)
    ssR = np.ascontiguousarray(ssR).astype(bf)

    trim = np.where(
        np.arange(128)[:, None] > np.arange(128)[None, :], -1e30, 0.0
    ).astype(np.float32)
    onec = np.ones((128, 1), dtype=bf)
    oner = np.ones((1, 128), dtype=np.float32)
    iden = np.eye(128, dtype=bf)

    in_maps = []
    for c in range(NC):
        wqkv = np.concatenate(
            [
                wqTp[:, 4 * c: 4 * c + 4].reshape(D, 512),
                wkTp[:, c],
                wvT[:, c],
            ],
            axis=1,
        ).astype(bf)
        in_maps.append(
            {
                "xT": xT,
                "wqkvT": np.ascontiguousarray(wqkv),
                "woT": woT,
                "ccR": ccR,
                "ssR": ssR,
                "trim": trim,
                "onec": onec,
                "oner": oner,
                "iden": iden,
            }
        )

    res = bass_utils.run_bass_kernel_spmd(
        nc, in_maps, core_ids=list(range(NC)), trace=PROFILE, tmpdir=TMPDIR
    )
    if PROFILE:
        print(f"HW exec time: {res.exec_time_ns} ns")
        if res.instructions_and_trace is not None:
            print(f"trace: {res.instructions_and_trace[1]}")

    out_full = np.empty((BS, D), dtype=np.float32)
    for c in range(NC):
        out_full[R * c: R * (c + 1)] = res.results[c]["out"]
    return out_full.reshape(B, S, D)


# revision 37
# speedup vs baseline: 1.5944x; 1.1329x over previous
"""Distributed Trainium2 attention kernel (8 NeuronCores).

Strategy: tensor-parallel over heads for QKV projection + attention
(4 query heads + their 1 shared KV head per core, identical causal loop
structure on every core), then an AllToAll switches to row-sharding so
each core computes the output projection for its 512 rows with the full
wo. Host reassembles rows. All matmuls run in bf16 with fp32 PSUM
accumulation; softmax runs unnormalized with the normalization folded in
after the PV matmul (per-head row sums via a ones-matmul).

RoPE is applied in row-major layout via a host-side even/odd column
permutation of wq/wk (rotation becomes contiguous half-block arithmetic),
then q/k are transposed to [head_dim, rows] on the TensorEngine for the
attention matmuls.
"""

import numpy as np
import ml_dtypes

import concourse.bass as bass
import concourse.mybir as mybir
import concourse.tile as tile
from concourse import bacc
from concourse import bass_utils

B, S, D = 2, 2048, 4096
H, HKV, HD = 32, 8, 128
HD2 = HD // 2
NC = 8
HL = H // NC            # 4 local q heads per core
BS = B * S              # 4096 global rows
R = BS // NC            # 512 output rows per core
NRB = BS // 128         # 32 row blocks
NDT = D // 128          # 32 contraction tiles
SCALE = 1.0 / float(np.sqrt(HD))
BF = mybir.dt.bfloat16
F32 = mybir.dt.float32

PROFILE = False         # set by test.py for neuron-profile capture
TMPDIR = None           # set by test.py to keep the trace dir


def _emit(nc, tc, io):
    xT, wqkvT, woT, ccR, ssR, trim, onec, oner, iden, out = io

    with (
        tc.tile_pool(name="ps", bufs=8, space="PSUM") as ps,
        tc.tile_pool(name="wbuf", bufs=1) as wbuf,
        tc.tile_pool(name="qbuf", bufs=1) as qbuf,
        tc.tile_pool(name="kvbuf", bufs=1) as kvbuf,
        tc.tile_pool(name="abuf", bufs=1) as abuf,
        tc.tile_pool(name="cbuf", bufs=1) as cbuf,
        tc.tile_pool(name="xs", bufs=10) as xs,
        tc.tile_pool(name="cs", bufs=6) as cs,
        tc.tile_pool(name="es", bufs=8) as es,
        tc.tile_pool(name="ws", bufs=16) as ws,
        tc.tile_pool(name="ts", bufs=8) as ts,
        tc.tile_pool(name="ans", bufs=6) as ans,
        tc.tile_pool(name="rsp", bufs=4) as rsp,
        tc.tile_pool(name="os", bufs=3) as osp,
        tc.tile_pool(name="dram", bufs=1, space="DRAM") as dram,
    ):
        # ---- constants ----
        trim_sb = cbuf.tile([128, 128], F32, tag="tm")
        nc.sync.dma_start(trim_sb[:], trim[:])
        onec_sb = cbuf.tile([128, 1], BF, tag="oc")
        nc.sync.dma_start(onec_sb[:], onec[:])
        oner_sb = cbuf.tile([1, 128], F32, tag="orr")
        nc.sync.dma_start(oner_sb[:], oner[:])
        iden_sb = cbuf.tile([128, 128], BF, tag="idn")
        nc.sync.dma_start(iden_sb[:], iden[:])

        # resident QKV weights: col = dt*768 + [0:512 q | 512:640 k | 640:768 v]
        w_sb = wbuf.tile([128, NDT * 768], BF, tag="w")
        for dt in range(NDT):
            nc.sync.dma_start(
                w_sb[:, dt * 768: dt * 768 + 768],
                wqkvT[dt * 128: (dt + 1) * 128, :],
            )

        q_sb = qbuf.tile([128, HL * BS], BF, tag="q")     # col = h*4096 + row
        kT_sb = kvbuf.tile([128, BS], BF, tag="k")        # col = row
        v_sb = kvbuf.tile([128, BS], BF, tag="v")         # col = rb*128 + hd

        # Two half-size AllToAlls: heads {0,1} and heads {2,3} per core.
        # The first fires mid-attention (its writers finish early), hiding
        # its latency; phase D starts accumulating on the first half while
        # the second is still in flight.
        a2a_in1 = dram.tile([BS // 2, R], BF, name="a2a_in1")
        a2a_out1 = dram.tile([BS // 2, R], BF, name="a2a_out1")
        a2a_in2 = dram.tile([BS // 2, R], BF, name="a2a_in2")
        a2a_out2 = dram.tile([BS // 2, R], BF, name="a2a_out2")

        # ---- phase B: QKV projection + RoPE + transposes ----
        # The rope+transpose tail of row block rb is emitted one iteration
        # late, behind rb+1's matmuls, so the PE queue never stalls on the
        # DVE rope chain.
        def b_rope_tail_q(rb, ps_q):
            cct = cs.tile([128, 256], BF, tag="cc")
            nc.sync.dma_start(cct[:], ccR[:, rb * 256: (rb + 1) * 256])
            sst = cs.tile([128, 256], BF, tag="ss")
            nc.sync.dma_start(sst[:], ssR[:, rb * 256: (rb + 1) * 256])

            # q rotation, all 4 heads at once via strided APs
            qe = ps_q[:].rearrange("p (h d) -> p h d", d=128)[:, :, 0:HD2]
            qo = ps_q[:].rearrange("p (h d) -> p h d", d=128)[:, :, HD2:HD]
            t1 = ts.tile([128, 256], BF, tag="t")
            t2 = ts.tile([128, 256], BF, tag="t")
            t3 = ts.tile([128, 256], BF, tag="t")
            t4 = ts.tile([128, 256], BF, tag="t")
            nc.vector.tensor_mul(t1[:], qe, cct[:])
            nc.vector.tensor_mul(t2[:], qo, sst[:])
            nc.vector.tensor_mul(t3[:], qe, sst[:])
            nc.vector.tensor_mul(t4[:], qo, cct[:])
            qrot = ts.tile([128, 512], BF, tag="qr")
            qre = qrot[:].rearrange("p (h d) -> p h d", d=128)[:, :, 0:HD2]
            qro = qrot[:].rearrange("p (h d) -> p h d", d=128)[:, :, HD2:HD]
            nc.vector.tensor_sub(qre, t1[:], t2[:])
            nc.vector.tensor_add(qro, t3[:], t4[:])
            return (qrot,)

        def b_transpose_tail_q(rb, qrot):
            # transpose q (4 heads, packed into one psum bank)
            ps_tq = ps.tile([128, 512], BF, tag="b")
            for h in range(HL):
                nc.tensor.transpose(
                    ps_tq[:, h * 128: (h + 1) * 128],
                    qrot[:, h * 128: (h + 1) * 128],
                    iden_sb[:],
                )
            q_dst = (
                q_sb[:]
                .rearrange("p (h r) -> p h r", h=HL)
                [:, :, rb * 128: (rb + 1) * 128]
            )
            nc.vector.tensor_copy(
                q_dst, ps_tq[:].rearrange("p (h r) -> p h r", h=HL)
            )

        def b_rope_tail_kv(rb, ps_kv):
            cct = cs.tile([128, 256], BF, tag="cc")
            nc.sync.dma_start(cct[:], ccR[:, rb * 256: (rb + 1) * 256])
            sst = cs.tile([128, 256], BF, tag="ss")
            nc.sync.dma_start(sst[:], ssR[:, rb * 256: (rb + 1) * 256])

            # k rotation (one head)
            ke = ps_kv[:, 0:HD2]
            ko = ps_kv[:, HD2:HD]
            u1 = ts.tile([128, 64], BF, tag="u")
            u2 = ts.tile([128, 64], BF, tag="u")
            u3 = ts.tile([128, 64], BF, tag="u")
            u4 = ts.tile([128, 64], BF, tag="u")
            nc.vector.tensor_mul(u1[:], ke, cct[:, 0:HD2])
            nc.vector.tensor_mul(u2[:], ko, sst[:, 0:HD2])
            nc.vector.tensor_mul(u3[:], ke, sst[:, 0:HD2])
            nc.vector.tensor_mul(u4[:], ko, cct[:, 0:HD2])
            krot = ts.tile([128, 128], BF, tag="kr")
            nc.vector.tensor_sub(krot[:, 0:HD2], u1[:], u2[:])
            nc.vector.tensor_add(krot[:, HD2:HD], u3[:], u4[:])

            # v: plain copy to row-major storage
            nc.scalar.activation(
                v_sb[:, rb * 128: (rb + 1) * 128], ps_kv[:, 128:256],
                mybir.ActivationFunctionType.Copy,
            )
            return (krot,)

        def b_transpose_tail_kv(rb, krot):
            ps_tk = ps.tile([128, 128], BF, tag="b")
            nc.tensor.transpose(ps_tk[:], krot[:], iden_sb[:])
            nc.vector.tensor_copy(kT_sb[:, rb * 128: (rb + 1) * 128], ps_tk[:])

        pending = None
        rot = None
        for rb in range(NRB):
            ps_q = ps.tile([128, 512], F32, tag="b")      # [rows, 4 q heads]
            ps_kv = ps.tile([128, 256], F32, tag="b")     # [rows, k|v]
            for dt in range(NDT):
                xt = xs.tile([128, 128], BF, tag="x")
                eng = (nc.gpsimd, nc.scalar, nc.sync)[dt % 3]
                eng.dma_start(
                    xt[:], xT[dt * 128: (dt + 1) * 128, rb * 128: (rb + 1) * 128]
                )
                st, sp = dt == 0, dt == NDT - 1
                nc.tensor.matmul(
                    ps_q[:], xt[:], w_sb[:, dt * 768: dt * 768 + 512],
                    start=st, stop=sp,
                )
                nc.tensor.matmul(
                    ps_kv[:], xt[:], w_sb[:, dt * 768 + 512: dt * 768 + 768],
                    start=st, stop=sp,
                )
                if dt == 2 and pending is not None:
                    rot = (pending[0],) + b_rope_tail_q(pending[0], pending[1]) \
                        + b_rope_tail_kv(pending[0], pending[2])
                    pending = None
                if dt == 12 and rot is not None:
                    b_transpose_tail_q(rot[0], rot[1])
                    b_transpose_tail_kv(rot[0], rot[2])
                    rot = None
            pending = (rb, ps_q, ps_kv)
        rot = (pending[0],) + b_rope_tail_q(pending[0], pending[1]) \
            + b_rope_tail_kv(pending[0], pending[2])
        b_transpose_tail_q(rot[0], rot[1])
        b_transpose_tail_kv(rot[0], rot[2])

        # ---- phase C: causal attention, paired interleaved chains ----
        # Each (b, h, ci) is an independent chain; two chains are emitted
        # interleaved so one chain's exp latency hides under the other's
        # matmuls. Pairing ci=0 with ci=3 (and 1 with 2) balances lengths.
        def attn_chain(b, h, ci):
            qbase = h * BS + b * S
            ps_attn = ps.tile([128, 512], F32, tag="b", name=f"pa{b}{h}{ci}")
            ps_rs = ps.tile([1, 512], F32, tag="b", name=f"pr{b}{h}{ci}")
            jmax = 4 * ci + 3

            def qspan(j):
                q0 = max(j * 128, 512 * ci)
                return q0, 512 * ci + 512 - q0

            def scores(j):
                q0, w = qspan(j)
                kcol = (b * 16 + j) * 128
                ps_s = ps.tile([128, 512], F32, tag="b", name=f"s{j}")
                nc.tensor.matmul(
                    ps_s[:, 0:w],
                    kT_sb[:, kcol: kcol + 128],
                    q_sb[:, qbase + q0: qbase + q0 + w],
                    start=True, stop=True,
                )
                if j // 4 == ci:
                    nc.vector.tensor_add(
                        ps_s[:, 0:128], ps_s[:, 0:128], trim_sb[:]
                    )
                et = es.tile([128, 512], BF, tag="e", name=f"e{j}")
                nc.scalar.activation(
                    et[:, 0:w], ps_s[:, 0:w],
                    mybir.ActivationFunctionType.Exp, scale=SCALE,
                )
                return et

            def pv(j, et):
                q0, w = qspan(j)
                off = q0 - 512 * ci
                kcol = (b * 16 + j) * 128
                nc.tensor.matmul(
                    ps_attn[:, off: off + w],
                    v_sb[:, kcol: kcol + 128],
                    et[:, 0:w],
                    start=(j == 0), stop=(j == jmax),
                )
                nc.tensor.matmul(
                    ps_rs[:, off: off + w],
                    onec_sb[:],
                    et[:, 0:w],
                    start=(j == 0), stop=(j == jmax),
                )

            prev = None
            for j in range(jmax + 1):
                et = scores(j)
                if prev is not None:
                    pv(prev[0], prev[1])
                prev = (j, et)
                yield
            pv(prev[0], prev[1])
            rc = rsp.tile([1, 512], F32, tag="rc")
            nc.vector.reciprocal(rc[:], ps_rs[:])
            bc_sb = rsp.tile([128, 512], F32, tag="bcs")
            nc.gpsimd.partition_broadcast(bc_sb[:], rc[:])
            an = ans.tile([128, 512], BF, tag="an")
            nc.vector.tensor_mul(an[:], ps_attn[:], bc_sb[:])
            dst = a2a_in1 if h < 2 else a2a_in2
            blk = 256 * (b * 4 + ci) + 128 * (h % 2)
            nc.sync.dma_start(dst[blk: blk + 128, :], an[:])
            yield

        # Continuous worklist: always two chains in flight, staggered phases.
        # Heads 0,1 first (both batches) so the first AllToAll's inputs
        # complete halfway through the attention phase.
        todo = []
        for h01 in ((0, 1), (2, 3)):
            for b in range(B):
                for h in h01:
                    todo += [(b, h, 0), (b, h, 3), (b, h, 1), (b, h, 2)]
        todo.reverse()
        active = [attn_chain(*todo.pop()), attn_chain(*todo.pop())]
        while active:
            for g in list(active):
                if next(g, StopIteration) is StopIteration:
                    active.remove(g)
                    if todo:
                        active.append(attn_chain(*todo.pop()))

        nc.gpsimd.collective_compute(
            "AllToAll",
            mybir.AluOpType.bypass,
            replica_groups=[list(range(NC))],
            ins=[a2a_in1.opt()],
            outs=[a2a_out1.opt()],
        )
        nc.gpsimd.collective_compute(
            "AllToAll",
            mybir.AluOpType.bypass,
            replica_groups=[list(range(NC))],
            ins=[a2a_in2.opt()],
            outs=[a2a_out2.opt()],
        )

        # ---- phase D: output projection for this core's 512 rows ----
        # ht order: first the tiles delivered by the first AllToAll
        # (head%4 in {0,1}), so accumulation overlaps the second one.
        ht_order = [4 * i + l for l in (0, 1) for i in range(8)] + \
                   [4 * i + l for l in (2, 3) for i in range(8)]
        at_sb = abuf.tile([128, 32 * 512], BF, tag="at")  # col = ht*512 + row
        for ht in ht_order:
            i, htl = ht // 4, ht % 4
            src = a2a_out1 if htl < 2 else a2a_out2
            srow = (i * 2 + (htl % 2)) * 128
            nc.sync.dma_start(
                at_sb[:, ht * 512: (ht + 1) * 512],
                src[srow: srow + 128, :],
            )
        for cg in range(8):
            po = [
                ps.tile([128, 512], F32, tag="b", name=f"po{cg}_{i}")
                for i in range(4)
            ]
            for n_ht, ht in enumerate(ht_order):
                wt = ws.tile([128, 512], BF, tag="wo")
                weng = (nc.sync, nc.scalar, nc.gpsimd)[ht % 3]
                weng.dma_start(
                    wt[:], woT[ht * 128: (ht + 1) * 128, cg * 512: (cg + 1) * 512]
                )
                for rt in range(4):
                    nc.tensor.matmul(
                        po[rt][:],
                        at_sb[:, ht * 512 + rt * 128: ht * 512 + (rt + 1) * 128],
                        wt[:],
                        start=(n_ht == 0), stop=(n_ht == 31),
                    )
            for rt in range(4):
                ot = osp.tile([128, 512], F32, tag="o")
                nc.vector.tensor_copy(ot[:], po[rt][:])
                nc.sync.dma_start(
                    out[rt * 128: (rt + 1) * 128, cg * 512: (cg + 1) * 512], ot[:]
                )


_LDW_PATCHED = False


def _patch_ldw_opt():
    """Enable walrus's redundant-LDWEIGHTS elision (off by default in
    concourse's compile flags; our phase-B matmul pairs share the same
    stationary operand back to back)."""
    global _LDW_PATCHED
    if _LDW_PATCHED:
        return
    _LDW_PATCHED = True
    real_run = bass_utils.run_command

    def run_hook(argv, **kw):
        argv = [
            a.replace("--enable-ldw-opt=false", "--enable-ldw-opt=true")
            if isinstance(a, str) else a
            for a in argv
        ]
        return real_run(argv, **kw)

    bass_utils.run_command = run_hook


def _build():
    nc = bacc.Bacc("TRN2", target_bir_lowering=False, debug=False, num_devices=NC)
    xT = nc.dram_tensor("xT", [D, BS], BF, kind="ExternalInput")
    wqkvT = nc.dram_tensor("wqkvT", [D, 768], BF, kind="ExternalInput")
    woT = nc.dram_tensor("woT", [D, D], BF, kind="ExternalInput")
    ccR = nc.dram_tensor("ccR", [128, NRB * 256], BF, kind="ExternalInput")
    ssR = nc.dram_tensor("ssR", [128, NRB * 256], BF, kind="ExternalInput")
    trim = nc.dram_tensor("trim", [128, 128], F32, kind="ExternalInput")
    onec = nc.dram_tensor("onec", [128, 1], BF, kind="ExternalInput")
    oner = nc.dram_tensor("oner", [1, 128], F32, kind="ExternalInput")
    iden = nc.dram_tensor("iden", [128, 128], BF, kind="ExternalInput")
    out = nc.dram_tensor("out", [R, D], F32, kind="ExternalOutput")
    with tile.TileContext(nc) as tc:
        _emit(nc, tc, (xT, wqkvT, woT, ccR, ssR, trim, onec, oner, iden, out))
    nc.compile()
    return nc


_NC = None


def kernel(x, wq, wk, wv, wo, freqs_cos, freqs_sin, mask, start_pos):
    global _NC
    if _NC is None:
        _NC = _build()
    nc = _NC
    bf = ml_dtypes.bfloat16

    x = np.asarray(x, dtype=np.float32)
    xT = np.ascontiguousarray(x.reshape(BS, D).T).astype(bf)

    perm = np.concatenate([np.arange(0, HD, 2), np.arange(1, HD, 2)])
    wqTp = np.asarray(wq, np.float32).T.reshape(D, H, HD)[:, :, perm]
    wkTp = np.asarray(wk, np.float32).T.reshape(D, HKV, HD)[:, :, perm]
    wvT = np.asarray(wv, np.float32).T.reshape(D, HKV, HD)
    woT = np.ascontiguousarray(np.asarray(wo, np.float32).T).astype(bf)

    fc = np.asarray(freqs_cos, np.float32)
    fs = np.asarray(freqs_sin, np.float32)
    # row-major RoPE tables per row block, replicated x4 along free axis
    pos = (np.arange(BS) % S).reshape(NRB, 128)
    ccR = np.tile(fc[pos], (1, 1, 4)).transpose(1, 0, 2).reshape(128, NRB * 256)
    ssR = np.tile(fs[pos], (1, 1, 4)).transpose(1, 0, 2).reshape(128, NRB * 256)
    ccR = np.ascontiguousarray(ccR).astype(bf)
    ssR = np.ascontiguousarray(ssR).astype(bf)

    trim = np.where(
        np.arange(128)[:, None] > np.arange(128)[None, :], -1e30, 0.0
    ).astype(np.float32)
    onec = np.ones((128, 1), dtype=bf)
    oner = np.ones((1, 128), dtype=np.float32)
    iden = np.eye(128, dtype=bf)

    in_maps = []
    for c in range(NC):
        wqkv = np.concatenate(
            [
                wqTp[:, 4 * c: 4 * c + 4].reshape(D, 512),
                wkTp[:, c],
                wvT[:, c],
            ],
            axis=1,
        ).astype(bf)
        in_maps.append(
            {
                "xT": xT,
                "wqkvT": np.ascontiguousarray(wqkv),
                "woT": woT,
                "ccR": ccR,
                "ssR": ssR,
                "trim": trim,
                "onec": onec,
                "oner": oner,
                "iden": iden,
            }
        )

    res = bass_utils.run_bass_kernel_spmd(
        nc, in_maps, core_ids=list(range(NC)), trace=PROFILE, tmpdir=TMPDIR
    )
    if PROFILE:
        print(f"HW exec time: {res.exec_time_ns} ns")
        if res.instructions_and_trace is not None:
            print(f"trace: {res.instructions_and_trace[1]}")

    out_full = np.empty((BS, D), dtype=np.float32)
    for c in range(NC):
        out_full[R * c: R * (c + 1)] = res.results[c]["out"]
    return out_full.reshape(B, S, D)
